# revision 16
# baseline (speedup 1.0000x reference)
"""Trainium2 Bass kernel for nn_AttnBlock (GroupNorm + single-head attention
block over [b=4, c=512, l=4096] fp32, 8 NeuronCores).

Sharding: core = (batch, query-half). Each core gets one batch item with its
query half permuted to columns 0..2047 (GroupNorm/attention are invariant to
a consistent permutation of l), computes the full block for its 2048 query
positions, and the host reassembles the [4, 512, 4096] output.

On-chip per core:
  - GroupNorm stats streamed from HBM (sum via DVE reduce, sum-sq via ACT
    Square+accum), group combine + broadcast via tiny TensorE matmuls with
    group-indicator matrices.
  - GN is folded into the QKV weights (w' = w * diag(m), bias fixups), the
    1/sqrt(c) attention scale folded into wq'.
  - Q/K [c, l] and V^T [l, c] computed as bf16 matmuls (x streamed again).
  - S^T = K^T Q per (i-block 512, j-tile 128); softmax without max-subtract
    (|S| <= ~6 for this model, exp stays in fp32 range), exp on ACT -> bf16;
    row sums s[i] via ones-vector matmuls; O_u = vT.T @ expS^T accumulated
    over j; proj with rank-1 bias inject (bp''' (x) s); normalize by 1/s
    after proj; residual re-DMA'd + added; DMA out.
"""
import os
import sys
from contextlib import ExitStack

import numpy as np

sys.path.insert(0, "/opt/trn_rl_repo")

import concourse.bass as bass
import concourse.tile as tile
from concourse import bacc, mybir

F32 = mybir.dt.float32
BF16 = mybir.dt.bfloat16

B, C, L = 4, 512, 4096
NQ = L // 2          # queries per core
P = 128
CO = C // P          # 4 channel blocks
NJT = L // P         # 32 j-tiles
NIB = NQ // 512      # 4 i-blocks
NLC = L // 512       # 8 l-chunks
NG = 32              # groups
GSZ = C // NG        # 16 channels per group
GPP = P // GSZ       # 8 groups per 128 partitions
EPS = 1e-6
SCALE = float(C) ** -0.5


def build_program():
    nc = bacc.Bacc("TRN2")
    x_d = nc.declare_dram_parameter("x", [C, L], F32, isOutput=False)
    wq_d = nc.declare_dram_parameter("wqT", [C, C], F32, isOutput=False)
    wk_d = nc.declare_dram_parameter("wkT", [C, C], F32, isOutput=False)
    wv_d = nc.declare_dram_parameter("wvT", [C, C], F32, isOutput=False)
    wp_d = nc.declare_dram_parameter("wpT", [C, C], F32, isOutput=False)
    gns_d = nc.declare_dram_parameter("gn_scale", [C], F32, isOutput=False)
    gnb_d = nc.declare_dram_parameter("gn_bias", [C], F32, isOutput=False)
    bq_d = nc.declare_dram_parameter("bq", [C], F32, isOutput=False)
    bv_d = nc.declare_dram_parameter("bv", [C], F32, isOutput=False)
    bp_d = nc.declare_dram_parameter("bp", [C], F32, isOutput=False)
    gm_d = nc.declare_dram_parameter("gmat", [P, GPP], F32, isOutput=False)
    gt_d = nc.declare_dram_parameter("gtmat", [GPP, P], F32, isOutput=False)
    out_d = nc.declare_dram_parameter("out", [C, NQ], F32, isOutput=True)

    with tile.TileContext(nc) as tc:
        attn_block(tc, x_d, wq_d, wk_d, wv_d, wp_d, gns_d, gnb_d,
                   bq_d, bv_d, bp_d, gm_d, gt_d, out_d)
    nc.compile()
    return nc


def attn_block(tc, x_d, wq_d, wk_d, wv_d, wp_d, gns_d, gnb_d, bq_d, bv_d,
               bp_d, gm_d, gt_d, out_d):
    nc = tc.nc
    x_v = x_d.ap().rearrange("(o p) l -> p o l", p=P)
    out_v = out_d.ap().rearrange("(o p) i -> p o i", p=P)

    with ExitStack() as ctx:
        # ---- persistent pools (whole kernel) ----
        big = ctx.enter_context(tc.tile_pool(name="big", bufs=1))
        wbp = ctx.enter_context(tc.tile_pool(name="wbp", bufs=1))
        small = ctx.enter_context(tc.tile_pool(name="small", bufs=1))
        ps = ctx.enter_context(tc.tile_pool(name="ps", bufs=3, space="PSUM"))

        q_sb = big.tile([P, CO, NQ], BF16, tag="qsb")
        k_sb = big.tile([P, CO, L], BF16, tag="ksb")
        vt_sb = big.tile([P, NJT, C], BF16, tag="vtsb")
        wq_b = wbp.tile([P, CO, C], BF16, tag="wqb")
        wk_b = wbp.tile([P, CO, C], BF16, tag="wkb")
        wv_b = wbp.tile([P, CO, C], BF16, tag="wvb")
        wp_b = wbp.tile([P, CO, C], BF16, tag="wpb")

        gns = small.tile([P, CO], F32, tag="gns")
        gnb = small.tile([P, CO], F32, tag="gnb")
        bq_s = small.tile([P, CO], F32, tag="bqs")
        bv_s = small.tile([P, CO], F32, tag="bvs")
        for v_d, v_t in ((gns_d, gns), (gnb_d, gnb), (bq_d, bq_s), (bv_d, bv_s)):
            nc.sync.dma_start(out=v_t[:], in_=v_d.ap().rearrange(
                "(o p) -> p o", p=P))
        bp_s = small.tile([1, C], F32, tag="bps")
        nc.sync.dma_start(out=bp_s[:], in_=bp_d.ap().rearrange("(u c) -> u c", u=1))

        bq2 = small.tile([P, CO], F32, tag="bq2")
        bp3_b = small.tile([1, C], BF16, tag="bp3b")
        ones_p = small.tile([P, 1], BF16, tag="onesp")
        nc.vector.memset(ones_p, 1.0)
        ones_1 = small.tile([1, P], F32, tag="ones1")
        nc.vector.memset(ones_1, 1.0)

        # ================= prologue: stats + folded weights =================
        with ExitStack() as pctx:
            xf_pool = pctx.enter_context(tc.tile_pool(name="xfp", bufs=3))
            wf_pool = pctx.enter_context(tc.tile_pool(name="wfp", bufs=1))
            wfs_pool = pctx.enter_context(tc.tile_pool(name="wfsp", bufs=2))
            pro = pctx.enter_context(tc.tile_pool(name="pro", bufs=1))

            wq_f = wf_pool.tile([P, CO, C], F32, tag="wqf")
            wv_f = wf_pool.tile([P, CO, C], F32, tag="wvf")
            wq_v = wq_d.ap().rearrange("(o p) c -> p o c", p=P)
            wv_v = wv_d.ap().rearrange("(o p) c -> p o c", p=P)
            for o in range(CO):
                nc.sync.dma_start(out=wq_f[:, o, :], in_=wq_v[:, o, :])
                nc.sync.dma_start(out=wv_f[:, o, :], in_=wv_v[:, o, :])

            # ---- streamed GroupNorm stats (bn_stats/bn_aggr per channel) ----
            bnst = pro.tile([P, CO, NLC, 6], F32, tag="bnst")
            for lc in range(NLC):
                l0 = lc * 512
                xf = xf_pool.tile([P, CO, 512], F32, tag="xf")
                for o in range(CO):
                    nc.sync.dma_start(out=xf[:, o, :],
                                      in_=x_v[:, o, l0 : l0 + 512])
                    nc.vector.bn_stats(out=bnst[:, o, lc, :], in_=xf[:, o, :])
            mv = pro.tile([P, CO, 2], F32, tag="mv")
            for o in range(CO):
                nc.vector.bn_aggr(out=mv[:, o, :], in_=bnst[:, o, :, :])
            # st8[:, 0:CO] = per-channel mean, st8[:, CO:] = per-channel E[x^2]
            st8 = small.tile([P, 2 * CO], F32, tag="st8")
            nc.vector.tensor_copy(st8[:, 0:CO], mv[:, :, 0])
            nc.vector.tensor_mul(st8[:, CO : 2 * CO], mv[:, :, 0], mv[:, :, 0])
            nc.vector.tensor_add(st8[:, CO : 2 * CO], st8[:, CO : 2 * CO],
                                 mv[:, :, 1])

            # ---- group combine: G[p, j] = (p // 16 == j) / (16 * 4096) ----
            g_mat = pro.tile([P, GPP], F32, tag="gmat")
            nc.sync.dma_start(out=g_mat[:], in_=gm_d.ap())
            gstat_ps = ps.tile([GPP, 2 * CO], F32, tag="mm")
            nc.tensor.matmul(gstat_ps, lhsT=g_mat, rhs=st8, start=True, stop=True)
            gstat = pro.tile([GPP, 2 * CO], F32, tag="gstat")
            nc.vector.tensor_copy(gstat, gstat_ps)
            var8 = pro.tile([GPP, CO], F32, tag="var8")
            nc.vector.tensor_mul(var8, gstat[:, 0:CO], gstat[:, 0:CO])
            nc.vector.tensor_sub(var8, gstat[:, CO : 2 * CO], var8)
            eps_t = pro.tile([GPP, 1], F32, tag="eps")
            nc.vector.memset(eps_t, EPS)
            mr8 = pro.tile([GPP, 2 * CO], F32, tag="mr8")
            nc.vector.tensor_copy(mr8[:, 0:CO], gstat[:, 0:CO])
            nc.scalar.activation(out=mr8[:, CO : 2 * CO], in_=var8,
                                 func=mybir.ActivationFunctionType.Sqrt,
                                 bias=eps_t)
            nc.vector.reciprocal(mr8[:, CO : 2 * CO], mr8[:, CO : 2 * CO])
            gt_mat = pro.tile([GPP, P], F32, tag="gtmat")
            nc.sync.dma_start(out=gt_mat[:], in_=gt_d.ap())
            bc_ps = ps.tile([P, 2 * CO], F32, tag="mm")
            nc.tensor.matmul(bc_ps, lhsT=gt_mat, rhs=mr8, start=True, stop=True)
            m44 = small.tile([P, CO], F32, tag="m44")
            nc.vector.tensor_mul(m44, bc_ps[:, CO : 2 * CO], gns)
            a44 = pro.tile([P, CO], F32, tag="a44")
            nc.vector.tensor_mul(a44, bc_ps[:, 0:CO], m44)
            nc.vector.tensor_sub(a44, gnb, a44)
            mq44 = pro.tile([P, CO], F32, tag="mq44")
            nc.vector.tensor_scalar_mul(mq44, m44, SCALE)

            # ---- folded weights (f32 -> bf16 with per-in-channel scale) ----
            for o in range(CO):
                nc.scalar.activation(out=wq_b[:, o, :], in_=wq_f[:, o, :],
                                     func=mybir.ActivationFunctionType.Copy,
                                     scale=mq44[:, o : o + 1])
                nc.scalar.activation(out=wv_b[:, o, :], in_=wv_f[:, o, :],
                                     func=mybir.ActivationFunctionType.Copy,
                                     scale=m44[:, o : o + 1])
            for w_d, w_t, sc in ((wk_d, wk_b, m44), (wp_d, wp_b, None)):
                wfs = wfs_pool.tile([P, CO, C], F32, tag="wfs")
                w_v = w_d.ap().rearrange("(o p) c -> p o c", p=P)
                for o in range(CO):
                    nc.sync.dma_start(out=wfs[:, o, :], in_=w_v[:, o, :])
                for o in range(CO):
                    if sc is None:
                        nc.vector.tensor_copy(w_t[:, o, :], wfs[:, o, :])
                    else:
                        nc.scalar.activation(
                            out=w_t[:, o, :], in_=wfs[:, o, :],
                            func=mybir.ActivationFunctionType.Copy,
                            scale=sc[:, o : o + 1])

            # ---- bias fixups ----
            # bq'' = (bq + wq @ a) * scale ; bv'' = bv + wv @ a
            bv2 = pro.tile([P, CO], F32, tag="bv2")
            for dst, w_t, b_t, sc in ((bq2, wq_f, bq_s, SCALE),
                                      (bv2, wv_f, bv_s, 1.0)):
                for oc in range(CO):
                    mv_ps = ps.tile([P, 1], F32, tag="mm")
                    for cc in range(CO):
                        nc.tensor.matmul(mv_ps,
                                         lhsT=w_t[:, cc, oc * P : (oc + 1) * P],
                                         rhs=a44[:, cc : cc + 1],
                                         start=(cc == 0), stop=(cc == CO - 1))
                    nc.vector.tensor_add(dst[:, oc : oc + 1], mv_ps,
                                         b_t[:, oc : oc + 1])
                if sc != 1.0:
                    nc.vector.tensor_scalar_mul(dst, dst, sc)
            bv2_b = pro.tile([P, CO], BF16, tag="bv2b")
            nc.vector.tensor_copy(bv2_b, bv2)
            # bp''' = bp + wp'' @ bv''
            bp3_ps = ps.tile([1, C], F32, tag="mm")
            for cc in range(CO):
                nc.tensor.matmul(bp3_ps, lhsT=bv2_b[:, cc : cc + 1],
                                 rhs=wp_b[:, cc, :],
                                 start=(cc == 0), stop=(cc == CO - 1))
            bp3_f = pro.tile([1, C], F32, tag="bp3f")
            nc.vector.tensor_add(bp3_f, bp3_ps, bp_s)
            nc.vector.tensor_copy(bp3_b, bp3_f)

        # ================= Q / K / V^T (x streamed again) =================
        with ExitStack() as qctx:
            xf2_pool = qctx.enter_context(tc.tile_pool(name="xf2p", bufs=3))
            xbf_pool = qctx.enter_context(tc.tile_pool(name="xbfp", bufs=3))
            for lc in range(NLC):
                l0 = lc * 512
                xf = xf2_pool.tile([P, CO, 512], F32, tag="xf2")
                xb = xbf_pool.tile([P, CO, 512], BF16, tag="xb")
                for o in range(CO):
                    nc.sync.dma_start(out=xf[:, o, :],
                                      in_=x_v[:, o, l0 : l0 + 512])
                    nc.vector.tensor_copy(xb[:, o, :], xf[:, o, :])
                for oc in range(CO):
                    kp = ps.tile([P, 512], F32, tag="mm")
                    for cc in range(CO):
                        nc.tensor.matmul(kp,
                                         lhsT=wk_b[:, cc, oc * P : (oc + 1) * P],
                                         rhs=xb[:, cc, :],
                                         start=(cc == 0), stop=(cc == CO - 1))
                    nc.scalar.activation(out=k_sb[:, oc, l0 : l0 + 512], in_=kp,
                                         func=mybir.ActivationFunctionType.Copy)
                for jt in range(4):
                    vp = ps.tile([P, C], F32, tag="mm")
                    for cc in range(CO):
                        nc.tensor.matmul(vp,
                                         lhsT=xb[:, cc, jt * P : (jt + 1) * P],
                                         rhs=wv_b[:, cc, :],
                                         start=(cc == 0), stop=(cc == CO - 1))
                    nc.vector.tensor_copy(vt_sb[:, lc * 4 + jt, :], vp)
                if lc < NIB:
                    for oc in range(CO):
                        qp = ps.tile([P, 512], F32, tag="mm")
                        for cc in range(CO):
                            nc.tensor.matmul(
                                qp, lhsT=wq_b[:, cc, oc * P : (oc + 1) * P],
                                rhs=xb[:, cc, :],
                                start=(cc == 0), stop=(cc == CO - 1))
                        nc.vector.tensor_scalar_add(q_sb[:, oc, l0 : l0 + 512],
                                                    qp, bq2[:, oc : oc + 1])

        # ================= attention + proj per i-block =================
        with ExitStack() as actx:
            p_pool = actx.enter_context(tc.tile_pool(name="ppool", bufs=4))
            osb_pool = actx.enter_context(tc.tile_pool(name="osb", bufs=2))
            out_pool = actx.enter_context(tc.tile_pool(name="outp", bufs=4))
            res_pool = actx.enter_context(tc.tile_pool(name="resp", bufs=4))
            tiny = actx.enter_context(tc.tile_pool(name="tiny", bufs=2))
            ps_o = actx.enter_context(
                tc.tile_pool(name="pso", bufs=4, space="PSUM"))
            ps_s = actx.enter_context(
                tc.tile_pool(name="pss", bufs=1, space="PSUM"))

            for ib in range(NIB):
                i0 = ib * 512
                s_ps = ps_s.tile([1, 512], F32, tag="srow")
                o_ps = [ps_o.tile([P, 512], F32, tag="oacc", name=f"oacc{cc}")
                        for cc in range(CO)]
                for jt in range(NJT):
                    st_ps = ps.tile([P, 512], F32, tag="mm")
                    for cc in range(CO):
                        nc.tensor.matmul(st_ps,
                                         lhsT=k_sb[:, cc, jt * P : (jt + 1) * P],
                                         rhs=q_sb[:, cc, i0 : i0 + 512],
                                         start=(cc == 0), stop=(cc == CO - 1))
                    p_bf = p_pool.tile([P, 512], BF16, tag="pbf")
                    nc.scalar.activation(out=p_bf, in_=st_ps,
                                         func=mybir.ActivationFunctionType.Exp)
                    nc.tensor.matmul(s_ps, lhsT=ones_p, rhs=p_bf,
                                     start=(jt == 0), stop=(jt == NJT - 1))
                    for cc in range(CO):
                        nc.tensor.matmul(o_ps[cc],
                                         lhsT=vt_sb[:, jt, cc * P : (cc + 1) * P],
                                         rhs=p_bf,
                                         start=(jt == 0), stop=(jt == NJT - 1))
                s_f = tiny.tile([1, 512], F32, tag="sf")
                nc.vector.tensor_copy(s_f, s_ps)
                s_b = tiny.tile([1, 512], BF16, tag="sb")
                nc.vector.tensor_copy(s_b, s_f)
                rinv1 = tiny.tile([1, 512], F32, tag="rinv1")
                nc.vector.reciprocal(rinv1, s_f)
                rb_ps = ps.tile([P, 512], F32, tag="mm")
                nc.tensor.matmul(rb_ps, lhsT=ones_1, rhs=rinv1,
                                 start=True, stop=True)
                rinv_b = tiny.tile([P, 512], F32, tag="rinvb")
                nc.vector.tensor_copy(rinv_b, rb_ps)
                o_sb = osb_pool.tile([P, CO, 512], BF16, tag="osb")
                for cc in range(CO):
                    if cc % 2 == 0:
                        nc.vector.tensor_copy(o_sb[:, cc, :], o_ps[cc])
                    else:
                        nc.scalar.activation(
                            out=o_sb[:, cc, :], in_=o_ps[cc],
                            func=mybir.ActivationFunctionType.Copy)
                for oc in range(CO):
                    res = res_pool.tile([P, 512], F32, tag="res")
                    nc.sync.dma_start(out=res, in_=x_v[:, oc, i0 : i0 + 512])
                    pj_ps = ps.tile([P, 512], F32, tag="mm")
                    for cc in range(CO):
                        nc.tensor.matmul(pj_ps,
                                         lhsT=wp_b[:, cc, oc * P : (oc + 1) * P],
                                         rhs=o_sb[:, cc, :],
                                         start=(cc == 0), stop=False)
                    nc.tensor.matmul(pj_ps,
                                     lhsT=bp3_b[:, oc * P : (oc + 1) * P],
                                     rhs=s_b, start=False, stop=True)
                    out_t = out_pool.tile([P, 512], F32, tag="outt")
                    nc.vector.tensor_mul(out_t, pj_ps, rinv_b)
                    nc.vector.tensor_add(out_t, out_t, res)
                    nc.sync.dma_start(out=out_v[:, oc, i0 : i0 + 512], in_=out_t)


def kernel(**inputs):
    x = np.ascontiguousarray(np.asarray(inputs["x"], np.float32))
    args = {
        "wqT": np.ascontiguousarray(np.asarray(inputs["wq"], np.float32).T),
        "wkT": np.ascontiguousarray(np.asarray(inputs["wk"], np.float32).T),
        "wvT": np.ascontiguousarray(np.asarray(inputs["wv"], np.float32).T),
        "wpT": np.ascontiguousarray(np.asarray(inputs["wp"], np.float32).T),
        "gn_scale": np.asarray(inputs["gn_scale"], np.float32),
        "gn_bias": np.asarray(inputs["gn_bias"], np.float32),
        "bq": np.asarray(inputs["bq"], np.float32),
        "bv": np.asarray(inputs["bv"], np.float32),
        "bp": np.asarray(inputs["bp"], np.float32),
    }
    pidx = np.arange(P)
    gmat = (pidx[:, None] // GSZ == np.arange(GPP)[None, :]).astype(np.float32)
    args["gmat"] = np.ascontiguousarray(gmat / float(GSZ))
    args["gtmat"] = np.ascontiguousarray(gmat.T)
    in_maps = []
    for core in range(8):
        bi, half = core // 2, core % 2
        sl = slice(half * NQ, (half + 1) * NQ)
        other = slice((1 - half) * NQ, (2 - half) * NQ)
        xp = np.ascontiguousarray(
            np.concatenate([x[bi][:, sl], x[bi][:, other]], axis=1))
        in_maps.append({"x": xp, **args})

    from concourse.bass_utils import run_bass_kernel_spmd

    nc = build_program()
    trace = bool(int(os.environ.get("KERNEL_TRACE", "0")))
    res = run_bass_kernel_spmd(nc, in_maps, core_ids=list(range(8)),
                               trace=trace)
    kernel.last_results = res
    out = np.empty((B, C, L), np.float32)
    for core in range(8):
        bi, half = core // 2, core % 2
        out[bi][:, half * NQ : (half + 1) * NQ] = res.results[core]["out"]
    return out


# revision 20
# speedup vs baseline: 1.0674x; 1.0674x over previous
"""Trainium2 Bass kernel for nn_AttnBlock (GroupNorm + single-head attention
block over [b=4, c=512, l=4096] fp32, 8 NeuronCores).

Sharding: core = (batch, query-half). Each core gets one batch item with its
query half permuted to columns 0..2047 (GroupNorm/attention are invariant to
a consistent permutation of l), computes the full block for its 2048 query
positions, and the host reassembles the [4, 512, 4096] output.

On-chip per core:
  - GroupNorm stats streamed from HBM (sum via DVE reduce, sum-sq via ACT
    Square+accum), group combine + broadcast via tiny TensorE matmuls with
    group-indicator matrices.
  - GN is folded into the QKV weights (w' = w * diag(m), bias fixups), the
    1/sqrt(c) attention scale folded into wq'.
  - Q/K [c, l] and V^T [l, c] computed as bf16 matmuls (x streamed again).
  - S^T = K^T Q per (i-block 512, j-tile 128); softmax without max-subtract
    (|S| <= ~6 for this model, exp stays in fp32 range), exp on ACT -> bf16;
    row sums s[i] via ones-vector matmuls; O_u = vT.T @ expS^T accumulated
    over j; proj with rank-1 bias inject (bp''' (x) s); normalize by 1/s
    after proj; residual re-DMA'd + added; DMA out.
"""
import os
import sys
from contextlib import ExitStack

import numpy as np

sys.path.insert(0, "/opt/trn_rl_repo")

import concourse.bass as bass
import concourse.tile as tile
from concourse import bacc, mybir

F32 = mybir.dt.float32
BF16 = mybir.dt.bfloat16

B, C, L = 4, 512, 4096
NQ = L // 2          # queries per core
P = 128
CO = C // P          # 4 channel blocks
NJT = L // P         # 32 j-tiles
NIB = NQ // 512      # 4 i-blocks
NLC = L // 512       # 8 l-chunks
NG = 32              # groups
GSZ = C // NG        # 16 channels per group
GPP = P // GSZ       # 8 groups per 128 partitions
EPS = 1e-6
SCALE = float(C) ** -0.5


def build_program():
    nc = bacc.Bacc("TRN2")
    x_d = nc.declare_dram_parameter("x", [C, L], F32, isOutput=False)
    wq_d = nc.declare_dram_parameter("wqT", [C, C], F32, isOutput=False)
    wk_d = nc.declare_dram_parameter("wkT", [C, C], F32, isOutput=False)
    wv_d = nc.declare_dram_parameter("wvT", [C, C], F32, isOutput=False)
    wp_d = nc.declare_dram_parameter("wpT", [C, C], F32, isOutput=False)
    gns_d = nc.declare_dram_parameter("gn_scale", [C], F32, isOutput=False)
    gnb_d = nc.declare_dram_parameter("gn_bias", [C], F32, isOutput=False)
    bq_d = nc.declare_dram_parameter("bq", [C], F32, isOutput=False)
    bv_d = nc.declare_dram_parameter("bv", [C], F32, isOutput=False)
    bp_d = nc.declare_dram_parameter("bp", [C], F32, isOutput=False)
    gm_d = nc.declare_dram_parameter("gmat", [P, GPP], F32, isOutput=False)
    gt_d = nc.declare_dram_parameter("gtmat", [GPP, P], F32, isOutput=False)
    out_d = nc.declare_dram_parameter("out", [C, NQ], F32, isOutput=True)

    with tile.TileContext(nc) as tc:
        attn_block(tc, x_d, wq_d, wk_d, wv_d, wp_d, gns_d, gnb_d,
                   bq_d, bv_d, bp_d, gm_d, gt_d, out_d)
    nc.compile()
    return nc


def attn_block(tc, x_d, wq_d, wk_d, wv_d, wp_d, gns_d, gnb_d, bq_d, bv_d,
               bp_d, gm_d, gt_d, out_d):
    nc = tc.nc
    x_v = x_d.ap().rearrange("(o p) l -> p o l", p=P)
    out_v = out_d.ap().rearrange("(o p) i -> p o i", p=P)

    with ExitStack() as ctx:
        # ---- persistent pools (whole kernel) ----
        big = ctx.enter_context(tc.tile_pool(name="big", bufs=1))
        wbp = ctx.enter_context(tc.tile_pool(name="wbp", bufs=1))
        small = ctx.enter_context(tc.tile_pool(name="small", bufs=1))
        ps = ctx.enter_context(tc.tile_pool(name="ps", bufs=3, space="PSUM"))

        q_sb = big.tile([P, CO, NQ], BF16, tag="qsb")
        k_sb = big.tile([P, CO, L], BF16, tag="ksb")
        vt_sb = big.tile([P, NJT, C], BF16, tag="vtsb")
        wq_b = wbp.tile([P, CO, C], BF16, tag="wqb")
        wk_b = wbp.tile([P, CO, C], BF16, tag="wkb")
        wv_b = wbp.tile([P, CO, C], BF16, tag="wvb")
        wp_b = wbp.tile([P, CO, C], BF16, tag="wpb")

        gns = small.tile([P, CO], F32, tag="gns")
        gnb = small.tile([P, CO], F32, tag="gnb")
        bq_s = small.tile([P, CO], F32, tag="bqs")
        bv_s = small.tile([P, CO], F32, tag="bvs")
        for v_d, v_t in ((gns_d, gns), (gnb_d, gnb), (bq_d, bq_s), (bv_d, bv_s)):
            nc.sync.dma_start(out=v_t[:], in_=v_d.ap().rearrange(
                "(o p) -> p o", p=P))
        bp_s = small.tile([1, C], F32, tag="bps")
        nc.sync.dma_start(out=bp_s[:], in_=bp_d.ap().rearrange("(u c) -> u c", u=1))

        bq2 = small.tile([P, CO], F32, tag="bq2")
        bp3_b = small.tile([1, C], BF16, tag="bp3b")
        ones_p = small.tile([P, 1], BF16, tag="onesp")
        nc.vector.memset(ones_p, 1.0)
        ones_1 = small.tile([1, P], F32, tag="ones1")
        nc.vector.memset(ones_1, 1.0)

        # ================= prologue: stats + folded weights =================
        with ExitStack() as pctx:
            xf_pool = pctx.enter_context(tc.tile_pool(name="xfp", bufs=3))
            wf_pool = pctx.enter_context(tc.tile_pool(name="wfp", bufs=1))
            wfs_pool = pctx.enter_context(tc.tile_pool(name="wfsp", bufs=2))
            pro = pctx.enter_context(tc.tile_pool(name="pro", bufs=1))

            # ---- streamed GroupNorm stats (bn_stats/bn_aggr per channel) ----
            # (x DMAs first: they are the critical path to the folded weights)
            bnst = pro.tile([P, CO, NLC, 6], F32, tag="bnst")
            for lc in range(NLC):
                l0 = lc * 512
                xf = xf_pool.tile([P, CO, 512], F32, tag="xf")
                for o in range(CO):
                    nc.sync.dma_start(out=xf[:, o, :],
                                      in_=x_v[:, o, l0 : l0 + 512])
                    nc.vector.bn_stats(out=bnst[:, o, lc, :], in_=xf[:, o, :])
            wq_f = wf_pool.tile([P, CO, C], F32, tag="wqf")
            wv_f = wf_pool.tile([P, CO, C], F32, tag="wvf")
            wq_v = wq_d.ap().rearrange("(o p) c -> p o c", p=P)
            wv_v = wv_d.ap().rearrange("(o p) c -> p o c", p=P)
            for o in range(CO):
                nc.sync.dma_start(out=wq_f[:, o, :], in_=wq_v[:, o, :])
                nc.sync.dma_start(out=wv_f[:, o, :], in_=wv_v[:, o, :])
            mv = pro.tile([P, CO, 2], F32, tag="mv")
            for o in range(CO):
                nc.vector.bn_aggr(out=mv[:, o, :], in_=bnst[:, o, :, :])
            # st8[:, 0:CO] = per-channel mean, st8[:, CO:] = per-channel E[x^2]
            st8 = small.tile([P, 2 * CO], F32, tag="st8")
            nc.vector.tensor_copy(st8[:, 0:CO], mv[:, :, 0])
            nc.vector.tensor_mul(st8[:, CO : 2 * CO], mv[:, :, 0], mv[:, :, 0])
            nc.vector.tensor_add(st8[:, CO : 2 * CO], st8[:, CO : 2 * CO],
                                 mv[:, :, 1])

            # ---- group combine: G[p, j] = (p // 16 == j) / (16 * 4096) ----
            g_mat = pro.tile([P, GPP], F32, tag="gmat")
            nc.sync.dma_start(out=g_mat[:], in_=gm_d.ap())
            gstat_ps = ps.tile([GPP, 2 * CO], F32, tag="mm")
            nc.tensor.matmul(gstat_ps, lhsT=g_mat, rhs=st8, start=True, stop=True)
            gstat = pro.tile([GPP, 2 * CO], F32, tag="gstat")
            nc.vector.tensor_copy(gstat, gstat_ps)
            var8 = pro.tile([GPP, CO], F32, tag="var8")
            nc.vector.tensor_mul(var8, gstat[:, 0:CO], gstat[:, 0:CO])
            nc.vector.tensor_sub(var8, gstat[:, CO : 2 * CO], var8)
            eps_t = pro.tile([GPP, 1], F32, tag="eps")
            nc.vector.memset(eps_t, EPS)
            mr8 = pro.tile([GPP, 2 * CO], F32, tag="mr8")
            nc.vector.tensor_copy(mr8[:, 0:CO], gstat[:, 0:CO])
            sq8 = pro.tile([GPP, CO], F32, tag="sq8")
            nc.scalar.activation(out=sq8, in_=var8,
                                 func=mybir.ActivationFunctionType.Sqrt,
                                 bias=eps_t)
            rscr = pro.tile([GPP, CO], F32, tag="rscr")
            nc.vector.reciprocal_approx_accurate(mr8[:, CO : 2 * CO], sq8, rscr)
            gt_mat = pro.tile([GPP, P], F32, tag="gtmat")
            nc.sync.dma_start(out=gt_mat[:], in_=gt_d.ap())
            bc_ps = ps.tile([P, 2 * CO], F32, tag="mm")
            nc.tensor.matmul(bc_ps, lhsT=gt_mat, rhs=mr8, start=True, stop=True)
            m44 = small.tile([P, CO], F32, tag="m44")
            nc.vector.tensor_mul(m44, bc_ps[:, CO : 2 * CO], gns)
            a44 = pro.tile([P, CO], F32, tag="a44")
            nc.vector.tensor_mul(a44, bc_ps[:, 0:CO], m44)
            nc.vector.tensor_sub(a44, gnb, a44)
            mq44 = pro.tile([P, CO], F32, tag="mq44")
            nc.vector.tensor_scalar_mul(mq44, m44, SCALE)

            # ---- folded weights (f32 -> bf16 with per-in-channel scale) ----
            for o in range(CO):
                nc.scalar.activation(out=wq_b[:, o, :], in_=wq_f[:, o, :],
                                     func=mybir.ActivationFunctionType.Copy,
                                     scale=mq44[:, o : o + 1])
                nc.scalar.activation(out=wv_b[:, o, :], in_=wv_f[:, o, :],
                                     func=mybir.ActivationFunctionType.Copy,
                                     scale=m44[:, o : o + 1])
            for w_d, w_t, sc in ((wk_d, wk_b, m44), (wp_d, wp_b, None)):
                wfs = wfs_pool.tile([P, CO, C], F32, tag="wfs")
                w_v = w_d.ap().rearrange("(o p) c -> p o c", p=P)
                for o in range(CO):
                    nc.sync.dma_start(out=wfs[:, o, :], in_=w_v[:, o, :])
                for o in range(CO):
                    if sc is None:
                        nc.vector.tensor_copy(w_t[:, o, :], wfs[:, o, :])
                    else:
                        nc.scalar.activation(
                            out=w_t[:, o, :], in_=wfs[:, o, :],
                            func=mybir.ActivationFunctionType.Copy,
                            scale=sc[:, o : o + 1])

            # ---- bias fixups ----
            # bq'' = (bq + wq @ a) * scale ; bv'' = bv + wv @ a
            bv2 = pro.tile([P, CO], F32, tag="bv2")
            for dst, w_t, b_t, sc in ((bq2, wq_f, bq_s, SCALE),
                                      (bv2, wv_f, bv_s, 1.0)):
                for oc in range(CO):
                    mv_ps = ps.tile([P, 1], F32, tag="mm")
                    for cc in range(CO):
                        nc.tensor.matmul(mv_ps,
                                         lhsT=w_t[:, cc, oc * P : (oc + 1) * P],
                                         rhs=a44[:, cc : cc + 1],
                                         start=(cc == 0), stop=(cc == CO - 1))
                    nc.vector.tensor_add(dst[:, oc : oc + 1], mv_ps,
                                         b_t[:, oc : oc + 1])
                if sc != 1.0:
                    nc.vector.tensor_scalar_mul(dst, dst, sc)
            bv2_b = pro.tile([P, CO], BF16, tag="bv2b")
            nc.vector.tensor_copy(bv2_b, bv2)
            # bp''' = bp + wp'' @ bv''
            bp3_ps = ps.tile([1, C], F32, tag="mm")
            for cc in range(CO):
                nc.tensor.matmul(bp3_ps, lhsT=bv2_b[:, cc : cc + 1],
                                 rhs=wp_b[:, cc, :],
                                 start=(cc == 0), stop=(cc == CO - 1))
            bp3_f = pro.tile([1, C], F32, tag="bp3f")
            nc.vector.tensor_add(bp3_f, bp3_ps, bp_s)
            nc.vector.tensor_copy(bp3_b, bp3_f)

        # ================= Q / K / V^T (x streamed again) =================
        with ExitStack() as qctx:
            xf2_pool = qctx.enter_context(tc.tile_pool(name="xf2p", bufs=3))
            xbf_pool = qctx.enter_context(tc.tile_pool(name="xbfp", bufs=3))
            for lc in range(NLC):
                l0 = lc * 512
                xf = xf2_pool.tile([P, CO, 512], F32, tag="xf2")
                xb = xbf_pool.tile([P, CO, 512], BF16, tag="xb")
                for o in range(CO):
                    nc.sync.dma_start(out=xf[:, o, :],
                                      in_=x_v[:, o, l0 : l0 + 512])
                    nc.vector.tensor_copy(xb[:, o, :], xf[:, o, :])
                for oc in range(CO):
                    kp = ps.tile([P, 512], F32, tag="mm")
                    for cc in range(CO):
                        nc.tensor.matmul(kp,
                                         lhsT=wk_b[:, cc, oc * P : (oc + 1) * P],
                                         rhs=xb[:, cc, :],
                                         start=(cc == 0), stop=(cc == CO - 1))
                    nc.scalar.activation(out=k_sb[:, oc, l0 : l0 + 512], in_=kp,
                                         func=mybir.ActivationFunctionType.Copy)
                for jt in range(4):
                    vp = ps.tile([P, C], F32, tag="mm")
                    for cc in range(CO):
                        nc.tensor.matmul(vp,
                                         lhsT=xb[:, cc, jt * P : (jt + 1) * P],
                                         rhs=wv_b[:, cc, :],
                                         start=(cc == 0), stop=(cc == CO - 1))
                    nc.vector.tensor_copy(vt_sb[:, lc * 4 + jt, :], vp)
                if lc < NIB:
                    for oc in range(CO):
                        qp = ps.tile([P, 512], F32, tag="mm")
                        for cc in range(CO):
                            nc.tensor.matmul(
                                qp, lhsT=wq_b[:, cc, oc * P : (oc + 1) * P],
                                rhs=xb[:, cc, :],
                                start=(cc == 0), stop=(cc == CO - 1))
                        nc.vector.tensor_scalar_add(q_sb[:, oc, l0 : l0 + 512],
                                                    qp, bq2[:, oc : oc + 1])

        # ================= attention + proj per i-block =================
        with ExitStack() as actx:
            p_pool = actx.enter_context(tc.tile_pool(name="ppool", bufs=4))
            osb_pool = actx.enter_context(tc.tile_pool(name="osb", bufs=2))
            out_pool = actx.enter_context(tc.tile_pool(name="outp", bufs=4))
            res_pool = actx.enter_context(tc.tile_pool(name="resp", bufs=4))
            tiny = actx.enter_context(tc.tile_pool(name="tiny", bufs=2))
            ps_o = actx.enter_context(
                tc.tile_pool(name="pso", bufs=4, space="PSUM"))
            ps_s = actx.enter_context(
                tc.tile_pool(name="pss", bufs=1, space="PSUM"))

            for ib in range(NIB):
                i0 = ib * 512
                s_ps = ps_s.tile([1, 512], F32, tag="srow")
                o_ps = [ps_o.tile([P, 512], F32, tag="oacc", name=f"oacc{cc}")
                        for cc in range(CO)]
                for jt in range(NJT):
                    st_ps = ps.tile([P, 512], F32, tag="mm")
                    for cc in range(CO):
                        nc.tensor.matmul(st_ps,
                                         lhsT=k_sb[:, cc, jt * P : (jt + 1) * P],
                                         rhs=q_sb[:, cc, i0 : i0 + 512],
                                         start=(cc == 0), stop=(cc == CO - 1))
                    p_bf = p_pool.tile([P, 512], BF16, tag="pbf")
                    nc.scalar.activation(out=p_bf, in_=st_ps,
                                         func=mybir.ActivationFunctionType.Exp)
                    nc.tensor.matmul(s_ps, lhsT=ones_p, rhs=p_bf,
                                     start=(jt == 0), stop=(jt == NJT - 1))
                    for cc in range(CO):
                        nc.tensor.matmul(o_ps[cc],
                                         lhsT=vt_sb[:, jt, cc * P : (cc + 1) * P],
                                         rhs=p_bf,
                                         start=(jt == 0), stop=(jt == NJT - 1))
                s_f = tiny.tile([1, 512], F32, tag="sf")
                nc.vector.tensor_copy(s_f, s_ps)
                s_b = tiny.tile([1, 512], BF16, tag="sb")
                nc.vector.tensor_copy(s_b, s_f)
                rinv1 = tiny.tile([1, 512], F32, tag="rinv1")
                nc.vector.reciprocal_approx_fast(rinv1, s_f)
                rb_ps = ps.tile([P, 512], F32, tag="mm")
                nc.tensor.matmul(rb_ps, lhsT=ones_1, rhs=rinv1,
                                 start=True, stop=True)
                rinv_b = tiny.tile([P, 512], F32, tag="rinvb")
                nc.vector.tensor_copy(rinv_b, rb_ps)
                o_sb = osb_pool.tile([P, CO, 512], BF16, tag="osb")
                for cc in range(CO):
                    if cc % 2 == 0:
                        nc.vector.tensor_copy(o_sb[:, cc, :], o_ps[cc])
                    else:
                        nc.scalar.activation(
                            out=o_sb[:, cc, :], in_=o_ps[cc],
                            func=mybir.ActivationFunctionType.Copy)
                for oc in range(CO):
                    res = res_pool.tile([P, 512], F32, tag="res")
                    nc.sync.dma_start(out=res, in_=x_v[:, oc, i0 : i0 + 512])
                    pj_ps = ps_o.tile([P, 512], F32, tag="oacc",
                                      name=f"pj{oc}")
                    for cc in range(CO):
                        nc.tensor.matmul(pj_ps,
                                         lhsT=wp_b[:, cc, oc * P : (oc + 1) * P],
                                         rhs=o_sb[:, cc, :],
                                         start=(cc == 0), stop=False)
                    nc.tensor.matmul(pj_ps,
                                     lhsT=bp3_b[:, oc * P : (oc + 1) * P],
                                     rhs=s_b, start=False, stop=True)
                    out_t = out_pool.tile([P, 512], F32, tag="outt")
                    nc.vector.tensor_mul(out_t, pj_ps, rinv_b)
                    nc.vector.tensor_add(out_t, out_t, res)
                    nc.sync.dma_start(out=out_v[:, oc, i0 : i0 + 512], in_=out_t)


def kernel(**inputs):
    x = np.ascontiguousarray(np.asarray(inputs["x"], np.float32))
    args = {
        "wqT": np.ascontiguousarray(np.asarray(inputs["wq"], np.float32).T),
        "wkT": np.ascontiguousarray(np.asarray(inputs["wk"], np.float32).T),
        "wvT": np.ascontiguousarray(np.asarray(inputs["wv"], np.float32).T),
        "wpT": np.ascontiguousarray(np.asarray(inputs["wp"], np.float32).T),
        "gn_scale": np.asarray(inputs["gn_scale"], np.float32),
        "gn_bias": np.asarray(inputs["gn_bias"], np.float32),
        "bq": np.asarray(inputs["bq"], np.float32),
        "bv": np.asarray(inputs["bv"], np.float32),
        "bp": np.asarray(inputs["bp"], np.float32),
    }
    pidx = np.arange(P)
    gmat = (pidx[:, None] // GSZ == np.arange(GPP)[None, :]).astype(np.float32)
    args["gmat"] = np.ascontiguousarray(gmat / float(GSZ))
    args["gtmat"] = np.ascontiguousarray(gmat.T)
    in_maps = []
    for core in range(8):
        bi, half = core // 2, core % 2
        sl = slice(half * NQ, (half + 1) * NQ)
        other = slice((1 - half) * NQ, (2 - half) * NQ)
        xp = np.ascontiguousarray(
            np.concatenate([x[bi][:, sl], x[bi][:, other]], axis=1))
        in_maps.append({"x": xp, **args})

    from concourse.bass_utils import run_bass_kernel_spmd

    nc = build_program()
    trace = bool(int(os.environ.get("KERNEL_TRACE", "0")))
    res = run_bass_kernel_spmd(nc, in_maps, core_ids=list(range(8)),
                               trace=trace)
    kernel.last_results = res
    out = np.empty((B, C, L), np.float32)
    for core in range(8):
        bi, half = core // 2, core % 2
        out[bi][:, half * NQ : (half + 1) * NQ] = res.results[core]["out"]
    return out
